# revision 1
# baseline (speedup 1.0000x reference)
"""Trainium2 Bass kernel for nn_FCNetwork3D (batch-1 dense CNN+MLP).

Network: x[1,2264] -> 6x Conv3d(1,1,3,SAME)+ReLU on the 6x6x6 tail ->
concat -> normalize -> Linear(2264,4096)+tanh -> Linear(4096,4096)+tanh
-> Linear(4096,32) -> scale/shift.

Sharding (8 cores): tensor-parallel on the two wide Linears.
  L0 column-parallel: core k computes h0 block k [512] (weights pre-
    transposed+normalization-folded on host), tanh locally.
  AllGather h0 (2KB/core) on-device.
  L1 column-parallel: core k computes h1 block k [512], tanh locally.
  L2 row-parallel over h1 blocks: core k computes a partial [1,32]
    (out_scale folded into weights, bias/out_shift split /8 across
    cores); host unshard = sum of the 8 partials.
The tiny conv stack runs replicated on every core as 6 matvecs with
host-built [216,216] conv matrices (pure weight placement, im2col-style).

Matmuls use fp32r (fp32 rounded to 11-bit mantissa by the PE datapath)
for the two big streams: 1 cycle/row at N>=256 vs 4 for plain fp32.
The conv matvecs (N=1, illegal for fp32r) stay plain fp32.
"""

import numpy as np

import concourse.bass as bass
import concourse.mybir as mybir
import concourse.tile as tile
from concourse import bacc
from concourse import bass_utils

F32 = mybir.dt.float32
F32R = mybir.dt.float32r
AF = mybir.ActivationFunctionType

NCORES = 8
OBS, ACTD, H, VOX = 2264, 32, 4096, 216
XH = OBS - VOX            # 2048 (x head)
S = H // NCORES           # 512 (per-core block of the hidden dim)
KC0 = XH // 128           # 16 x-head K-chunks
KC1 = H // 128            # 32 h0 K-chunks


def build_nc(reps: int = 1, fake_gather: bool = False):
    """Build the per-core Bass program (identical on all 8 cores; data
    differs via per-core inputs). reps>1 unrolls the whole body for
    steady-state throughput measurement. fake_gather replaces the
    AllGather with a DRAM round-trip + an hrest input (single-core
    TimelineSim oracle)."""
    nc = bacc.Bacc("TRN2", target_bir_lowering=False, debug=False,
                   num_devices=1 if fake_gather else NCORES)

    xh_d = nc.dram_tensor("xh", [XH], F32, kind="ExternalInput")
    v0_d = nc.dram_tensor("v0", [VOX], F32, kind="ExternalInput")
    ct_d = nc.dram_tensor("ct", [6, VOX, VOX], F32, kind="ExternalInput")
    cb_d = nc.dram_tensor("cb", [6], F32, kind="ExternalInput")
    one_d = nc.dram_tensor("onec", [1], F32, kind="ExternalInput")
    a0_d = nc.dram_tensor("a0", [OBS + 1, S], F32, kind="ExternalInput")
    a1_d = nc.dram_tensor("a1", [H + 1, S], F32, kind="ExternalInput")
    a2_d = nc.dram_tensor("a2", [S + 1, ACTD], F32, kind="ExternalInput")
    hrest_d = (nc.dram_tensor("hrest", [H], F32, kind="ExternalInput")
               if fake_gather else None)
    y_d = nc.dram_tensor("y", [1, ACTD], F32, kind="ExternalOutput")

    with tile.TileContext(nc) as tc:
        with (
            tc.tile_pool(name="wp", bufs=6) as wp,
            tc.tile_pool(name="cp", bufs=24) as cp,
            tc.tile_pool(name="sp", bufs=4) as sp,
            tc.tile_pool(name="ps", bufs=2, space="PSUM") as ps,
            tc.tile_pool(name="psa", bufs=1, space="PSUM") as psa,
            tc.tile_pool(name="psb", bufs=1, space="PSUM") as psb,
            tc.tile_pool(name="dr", bufs=2, space="DRAM") as dr,
        ):
            for _ in range(reps):
                G = 8
                # ============ phase A: issue every DMA load up front ========
                # sync and scalar each feed one HWDGE ring; no compute waits
                # ever precede these dispatches, so both rings stream at full
                # rate from t=0. gpsimd is reserved for the AG/output chain.
                one_t = sp.tile([1, 1], F32R)
                nc.sync.dma_start(out=one_t[:],
                                  in_=one_d.ap().unsqueeze(-1).bitcast(F32R))
                one_f = sp.tile([1, 1], F32)
                nc.sync.dma_start(out=one_f[:], in_=one_d.ap().unsqueeze(-1))
                cbb = sp.tile([128, 6], F32)
                nc.sync.dma_start(out=cbb[:],
                                  in_=cb_d.ap().unsqueeze(0).to_broadcast((128, 6)))
                xt = sp.tile([128, KC0], F32R)
                nc.sync.dma_start(
                    out=xt[:],
                    in_=xh_d.ap().rearrange("(c p) -> p c", p=128).bitcast(F32R))
                vc0 = sp.tile([128, 1], F32)
                vc1 = sp.tile([88, 1], F32)
                nc.sync.dma_start(out=vc0[:], in_=v0_d[0:128].unsqueeze(-1))
                nc.sync.dma_start(out=vc1[:], in_=v0_d[128:VOX].unsqueeze(-1))
                # conv weights first on the scalar ring (conv chain starts first)
                cw = []
                for i in range(6):
                    w00 = cp.tile([128, 128], F32)
                    w10 = cp.tile([88, 128], F32)
                    w01 = cp.tile([128, 88], F32)
                    w11 = cp.tile([88, 88], F32)
                    nc.sync.dma_start(out=w00[:], in_=ct_d[i, 0:128, 0:128])
                    nc.sync.dma_start(out=w10[:], in_=ct_d[i, 128:VOX, 0:128])
                    nc.sync.dma_start(out=w01[:], in_=ct_d[i, 0:128, 128:VOX])
                    nc.sync.dma_start(out=w11[:], in_=ct_d[i, 128:VOX, 128:VOX])
                    cw.append((w00, w10, w01, w11))
                # a0 stream alternating sync/gpsimd (L0 consumes first)
                a0t = []
                for g in range(KC0 // G):
                    wt = wp.tile([128, G * S], F32R)
                    weng = nc.sync if g % 2 == 0 else nc.gpsimd
                    weng.dma_start(
                        out=wt[:].rearrange("p (j e) -> p j e", j=G),
                        in_=a0_d[g * G * 128:(g + 1) * G * 128, :]
                        .rearrange("(j p) e -> p j e", p=128).bitcast(F32R))
                    a0t.append(wt)
                w16 = wp.tile([128, S], F32R)
                nc.sync.dma_start(out=w16[:],
                                  in_=a0_d[XH:XH + 128, :].bitcast(F32R))
                w17 = wp.tile([88, S], F32R)
                nc.sync.dma_start(out=w17[:],
                                  in_=a0_d[XH + 128:OBS, :].bitcast(F32R))
                a0b = sp.tile([1, S], F32R)
                nc.sync.dma_start(out=a0b[:], in_=a0_d[OBS:OBS + 1, :].bitcast(F32R))
                # a1 stream alternating sync/gpsimd
                a1t = []
                for g in range(KC1 // G):
                    wt = wp.tile([128, G * S], F32R)
                    weng = nc.sync if g % 2 == 0 else nc.gpsimd
                    weng.dma_start(
                        out=wt[:].rearrange("p (j e) -> p j e", j=G),
                        in_=a1_d[g * G * 128:(g + 1) * G * 128, :]
                        .rearrange("(j p) e -> p j e", p=128).bitcast(F32R))
                    a1t.append(wt)
                a1b = sp.tile([1, S], F32R)
                nc.sync.dma_start(out=a1b[:], in_=a1_d[H:H + 1, :].bitcast(F32R))
                a2t = sp.tile([128, (S // 128) * ACTD], F32R)
                nc.sync.dma_start(
                    out=a2t[:].rearrange("p (j e) -> p j e", j=S // 128),
                    in_=a2_d[0:S, :]
                    .rearrange("(j p) e -> p j e", p=128).bitcast(F32R))
                a2b = sp.tile([1, ACTD], F32R)
                nc.sync.dma_start(out=a2b[:], in_=a2_d[S:S + 1, :].bitcast(F32R))

                # ============ phase B: compute ==============================
                # conv stack: 6 serial matvecs in plain fp32
                for i in range(6):
                    w00, w10, w01, w11 = cw[i]
                    pm0 = ps.tile([128, 1], F32)
                    pm1 = ps.tile([88, 1], F32)
                    nc.tensor.matmul(pm0[:], w00[:], vc0[:], start=True, stop=False)
                    nc.tensor.matmul(pm0[:], w10[:], vc1[:], start=False, stop=True)
                    nc.tensor.matmul(pm1[:], w01[:], vc0[:], start=True, stop=False)
                    nc.tensor.matmul(pm1[:], w11[:], vc1[:], start=False, stop=True)
                    last = (i == 5)
                    nv0 = sp.tile([128, 1], F32R if last else F32)
                    nv1 = sp.tile([88, 1], F32R if last else F32)
                    nc.scalar.activation(nv0[:], pm0[:], AF.Relu,
                                         bias=cbb[:, i:i + 1])
                    nc.scalar.activation(nv1[:], pm1[:], AF.Relu,
                                         bias=cbb[0:88, i:i + 1])
                    vc0, vc1 = nv0, nv1
                cv0, cv1 = vc0, vc1

                # L0: h0_blk = tanh(xn @ A0 + b0_blk)  [1, 512]
                ph0 = psa.tile([1, S], F32)
                for g in range(KC0 // G):
                    for j in range(G):
                        c = g * G + j
                        nc.tensor.matmul(ph0[:], xt[:, c:c + 1],
                                         a0t[g][:, j * S:(j + 1) * S],
                                         start=(c == 0), stop=False)
                nc.tensor.matmul(ph0[:], cv0[:], w16[:], start=False, stop=False)
                nc.tensor.matmul(ph0[:], cv1[:], w17[:], start=False, stop=False)
                nc.tensor.matmul(ph0[:], one_t[:], a0b[:], start=False, stop=True)
                h0s = sp.tile([1, S], F32)
                nc.scalar.activation(h0s[:], ph0[:], AF.Tanh)

                # AllGather h0 blocks -> full h0 [4096] (gpsimd chain)
                h0g = sp.tile([128, KC1], F32R)
                if fake_gather:
                    ccin = dr.tile([S], F32)
                    nc.scalar.dma_start(out=ccin[:], in_=h0s[:])
                    nc.gpsimd.dma_start(
                        out=h0g[:, 0:S // 128],
                        in_=ccin[:].rearrange("(c p) -> p c", p=128).bitcast(F32R))
                    nc.gpsimd.dma_start(
                        out=h0g[:, S // 128:KC1],
                        in_=hrest_d[S:H].rearrange("(c p) -> p c", p=128).bitcast(F32R))
                else:
                    ccin = dr.tile([S], F32)
                    ccout = dr.tile([H], F32)
                    nc.scalar.dma_start(out=ccin[:], in_=h0s[:])
                    nc.gpsimd.collective_compute(
                        "AllGather", mybir.AluOpType.bypass,
                        replica_groups=[list(range(NCORES))],
                        ins=[ccin[:].opt()], outs=[ccout[:].opt()])
                    nc.scalar.dma_start(
                        out=h0g[:],
                        in_=ccout[:].rearrange("(c p) -> p c", p=128).bitcast(F32R))

                # L1: h1_blk = tanh(h0 @ A1 + b1_blk)  [1, 512]
                ph1 = psa.tile([1, S], F32)
                for g in range(KC1 // G):
                    for j in range(G):
                        c = g * G + j
                        nc.tensor.matmul(ph1[:], h0g[:, c:c + 1],
                                         a1t[g][:, j * S:(j + 1) * S],
                                         start=(c == 0), stop=False)
                nc.tensor.matmul(ph1[:], one_t[:], a1b[:], start=False, stop=True)
                h1s = sp.tile([1, S], F32)
                nc.scalar.activation(h1s[:], ph1[:], AF.Tanh)

                # L2 partial: y_k = h1_blk @ A2_blk + bias'/8  [1, 32]
                # h1 row -> column chunks on the PE (mm with a [1,1] one)
                pth = psb.tile([128, S // 128], F32)
                for c in range(S // 128):
                    nc.tensor.matmul(pth[:, c:c + 1],
                                     h1s[:, c * 128:(c + 1) * 128], one_f[:],
                                     start=True, stop=True)
                h1g = sp.tile([128, S // 128], F32R)
                nc.scalar.copy(h1g[:], pth[:])
                py = psa.tile([1, ACTD], F32)
                for c in range(S // 128):
                    nc.tensor.matmul(py[:], h1g[:, c:c + 1],
                                     a2t[:, c * ACTD:(c + 1) * ACTD],
                                     start=(c == 0), stop=False)
                nc.tensor.matmul(py[:], one_t[:], a2b[:], start=False, stop=True)
                ys = sp.tile([1, ACTD], F32)
                nc.scalar.copy(ys[:], py[:])
                nc.scalar.dma_start(out=y_d[:, :], in_=ys[:])

    nc.compile()
    return nc


def _conv_matrix(w: np.ndarray) -> np.ndarray:
    """[216,216] dense matrix of a 3x3x3 SAME cross-correlation on a
    6x6x6 grid: C[o, i] such that y.flat = C @ v.flat."""
    w = np.asarray(w, dtype=np.float32).reshape(3, 3, 3)
    C = np.zeros((VOX, VOX), dtype=np.float32)
    idx = np.arange(6)
    for dz in (-1, 0, 1):
        for dy in (-1, 0, 1):
            for dx in (-1, 0, 1):
                zo, zi = idx[max(0, -dz):6 - max(0, dz)], idx[max(0, dz):6 - max(0, -dz)]
                yo, yi = idx[max(0, -dy):6 - max(0, dy)], idx[max(0, dy):6 - max(0, -dy)]
                xo, xi = idx[max(0, -dx):6 - max(0, dx)], idx[max(0, dx):6 - max(0, -dx)]
                o = (zo[:, None, None] * 36 + yo[None, :, None] * 6 + xo[None, None, :]).ravel()
                i = (zi[:, None, None] * 36 + yi[None, :, None] * 6 + xi[None, None, :]).ravel()
                C[o, i] = w[dz + 1, dy + 1, dx + 1]
    return C


def make_in_maps(inputs: dict) -> list[dict]:
    """Host-side layout prep + sharding: fold normalization into A0,
    out_scale/shift into A2, pre-transpose weights, build conv matrices."""
    f = np.float32
    x = np.asarray(inputs["x"], f)
    W0, b0 = np.asarray(inputs["W0"], f), np.asarray(inputs["b0"], f)
    W1, b1 = np.asarray(inputs["W1"], f), np.asarray(inputs["b1"], f)
    W2, b2 = np.asarray(inputs["W2"], f), np.asarray(inputs["b2"], f)
    in_shift = np.asarray(inputs["in_shift"], f)
    in_scale = np.asarray(inputs["in_scale"], f)
    out_shift = np.asarray(inputs["out_shift"], f)
    out_scale = np.asarray(inputs["out_scale"], f)

    sc = (1.0 / (in_scale.astype(np.float64) + 1e-8)).astype(f)       # [2264]
    A0 = (W0 * sc[None, :]).T.astype(f)                               # [2264, 4096]
    bias0 = (b0 - (in_shift * sc) @ W0.T).astype(f)                   # [4096]
    A1 = W1.T.astype(f)                                               # [4096, 4096]
    A2 = (W2.T * out_scale[None, :]).astype(f)                        # [4096, 32]
    bias2 = ((b2 * out_scale + out_shift) / NCORES).astype(f)         # [32]

    ct = np.stack([_conv_matrix(inputs[f"cw{i}"]).T for i in range(6)])  # [6,216,216]
    cb = np.array([np.asarray(inputs[f"cb{i}"], f).ravel()[0]
                   for i in range(6)], f)

    xh = np.ascontiguousarray(x.ravel()[:XH])
    v0 = np.ascontiguousarray(x.ravel()[XH:])
    onec = np.ones([1], f)

    in_maps = []
    for k in range(NCORES):
        blk = slice(k * S, (k + 1) * S)
        a0 = np.concatenate([A0[:, blk], bias0[blk][None, :]], axis=0)
        a1 = np.concatenate([A1[:, blk], b1[blk][None, :]], axis=0)
        a2 = np.concatenate([A2[blk, :], bias2[None, :]], axis=0)
        in_maps.append(dict(
            xh=xh, v0=v0, ct=ct, cb=cb, onec=onec,
            a0=np.ascontiguousarray(a0),
            a1=np.ascontiguousarray(a1),
            a2=np.ascontiguousarray(a2),
        ))
    return in_maps


_NC_CACHE: dict = {}


def kernel(**inputs) -> np.ndarray:
    if "nc" not in _NC_CACHE:
        _NC_CACHE["nc"] = build_nc(reps=1)
    nc = _NC_CACHE["nc"]
    in_maps = make_in_maps(inputs)
    res = bass_utils.run_bass_kernel_spmd(nc, in_maps,
                                          core_ids=list(range(NCORES)))
    y = np.sum([res.results[k]["y"] for k in range(NCORES)], axis=0)
    return y.astype(np.float32)



# revision 23
# speedup vs baseline: 242.0494x; 242.0494x over previous
"""Trainium2 Bass kernel for nn_FCNetwork3D (batch-1 dense CNN+MLP).

Network: x[1,2264] -> 6x Conv3d(1,1,3,SAME)+ReLU on the 6x6x6 tail ->
concat -> normalize -> Linear(2264,4096)+tanh -> Linear(4096,4096)+tanh
-> Linear(4096,32) -> scale/shift.

Sharding (8 cores): tensor-parallel on the two wide Linears.
  L0 column-parallel: core k computes h0 block k [512] (weights pre-
    transposed + normalization folded on host), tanh locally.
  AllGather h0 (1KB/core bf16) on-device.
  L1 column-parallel: core k computes h1 block k [512], tanh locally.
  L2 row-parallel over h1 blocks: core k computes a partial [1,32]
    (out_scale folded into weights, bias/out_shift split /8 across
    cores); host unshard = sum of the 8 partials.

All weights/activations are bf16 (fp32 PSUM accumulate): halves the HBM
traffic of the memory-bound weight stream; measured end-to-end max rel
err ~2.5e-3 vs the fp32 reference (gate 2e-2).

The conv stack uses the z-banded form of the 3x3x3 SAME conv on the
6x6x6 grid: 3 matrices [36,36] per layer (one per z-offset) acting on
v viewed as [36 xy-sites, 6 z-slices] — 45KB of conv weights instead
of 1.1MB of dense [216,216] matrices.

DMA plan: three rings (sync/HWDGE, scalar/HWDGE, gpsimd/SWDGE) carry
~2.2MB each in need-order, so on hardware (rings run concurrently at
~100GB/s each) the stream takes ~22us and a0 lands by ~8us.  The h0
gather comes back as a single [1,4096] row (one descriptor) and is
redistributed to K-chunk columns with PE row->col transposes, batched
between L1 matmul groups.  Tiny paced matmuls bridge the PE across the
gather window so the tensor clock never drops to a low p-state before
the L1 burst.
"""

import numpy as np

import concourse.bass as bass
import concourse.mybir as mybir
import concourse.tile as tile
from concourse import bacc
from concourse import bass_utils

F32 = mybir.dt.float32
BF16 = mybir.dt.bfloat16
AF = mybir.ActivationFunctionType

NCORES = 8
OBS, ACTD, H, VOX = 2264, 32, 4096, 216
XH = OBS - VOX            # 2048 (x head)
S = H // NCORES           # 512 (per-core block of the hidden dim)
KC0 = XH // 128           # 16 x-head K-chunks
KC1 = H // 128            # 32 h0 K-chunks
NS = 36                   # xy-sites per z-slice
NZ = 6                    # z-slices
CTW = 6 * 3 * NS          # packed banded conv width (648)

# pkb (bf16 [128, 22] pack) column map
PB_X = 0                  # [0:16)   x head, partition-major chunks
PB_V = 16                 # [16:22)  voxel tail as [36, 6]
PB_W = 22
# pkbias (bf16 [1, 1088] pack, partition 0) column map
BB_ONE = 0                # 1.0
BB_B0 = 32                # bias0 row [32:544)
BB_B1 = 544               # bias1 row [544:1056)
BB_B2 = 1056              # bias2 row [1056:1088)
BB_W = 1088

A0P = (6, 5, 5)           # a0 K-chunk split across the three rings
G1 = 4                    # K-chunks per a1 piece (8 pieces)


def build_nc(reps: int = 1, fake_gather: bool = False):
    """Build the per-core Bass program (identical on all 8 cores; data
    differs via per-core inputs). reps>1 unrolls the whole body.
    fake_gather replaces the AllGather with a DRAM round-trip
    (single-core TimelineSim oracle)."""
    nc = bacc.Bacc("TRN2", target_bir_lowering=False, debug=False,
                   num_devices=1 if fake_gather else NCORES)

    ctb_d = nc.dram_tensor("ctb", [NS, CTW], BF16, kind="ExternalInput")
    pkf_d = nc.dram_tensor("pkf", [NS, 6], F32, kind="ExternalInput")
    pkb_d = nc.dram_tensor("pkb", [128, PB_W], BF16, kind="ExternalInput")
    pkbias_d = nc.dram_tensor("pkbias", [1, BB_W], BF16, kind="ExternalInput")
    a0_d = nc.dram_tensor("a0", [XH, S], BF16, kind="ExternalInput")
    wtail_d = nc.dram_tensor("wtail", [NS, NZ * S], BF16, kind="ExternalInput")
    a1_d = nc.dram_tensor("a1", [H, S], BF16, kind="ExternalInput")
    a2_d = nc.dram_tensor("a2", [S, ACTD], BF16, kind="ExternalInput")
    y_d = nc.dram_tensor("y", [1, ACTD], F32, kind="ExternalOutput")

    scr = nc.alloc_sbuf_tensor("warm_scr", [1, 64], BF16)

    with tile.TileContext(nc) as tc:
        with (
            tc.tile_pool(name="wp", bufs=11) as wp,
            tc.tile_pool(name="cp", bufs=2) as cp,
            tc.tile_pool(name="sp", bufs=2) as sp,
            tc.tile_pool(name="psC", bufs=2, space="PSUM") as psC,
            tc.tile_pool(name="psA", bufs=1, space="PSUM") as psA,
            tc.tile_pool(name="psB", bufs=1, space="PSUM") as psB,
            tc.tile_pool(name="psT", bufs=2, space="PSUM") as psT,
            tc.tile_pool(name="wq5", bufs=1) as wq5,
            tc.tile_pool(name="wq6", bufs=1) as wq6,
            tc.tile_pool(name="wq7", bufs=1) as wq7,
            tc.tile_pool(name="dr", bufs=2, space="DRAM") as dr,
        ):
            wqs = [wq5, wq6, wq7]
            for _ in range(reps):
                # ======== phase A: stage the DMA streams ================
                # ring need-order; bulk round-robined across all 3 rings
                ctb = cp.tile([NS, CTW], BF16)
                nc.sync.dma_start(out=ctb[:], in_=ctb_d.ap())
                pkf = sp.tile([NS, 6], F32)
                nc.scalar.dma_start(out=pkf[:], in_=pkf_d.ap())
                pkbias = sp.tile([1, BB_W], BF16)
                nc.scalar.dma_start(out=pkbias[:], in_=pkbias_d.ap())
                pkb = sp.tile([128, PB_W], BF16)
                nc.gpsimd.dma_start(out=pkb[:], in_=pkb_d.ap())
                wtail = cp.tile([NS, NZ * S], BF16)
                nc.scalar.dma_start(out=wtail[:], in_=wtail_d.ap())

                one_b = pkbias[0:1, BB_ONE:BB_ONE + 1]

                # tensor-clock warmup from t=0: a self-paced PE<->DVE
                # chain on an uninitialized scratch tensor (no deps, so
                # it schedules immediately; values are never read).
                pwm = psT.tile([1, 64], F32, name="ptr2")
                wseed = sp.tile([1, 1], BF16, name="brb")
                nc.vector.tensor_copy(wseed[:], scr.ap()[0:1, 0:1])
                for w in range(6):
                    nc.tensor.matmul(pwm[:], wseed[:], scr.ap()[0:1, :],
                                     start=(w == 0), stop=(w == 5),
                                     skip_group_check=True)
                    if w < 5:
                        wseed = sp.tile([1, 1], BF16, name="brb")
                        nc.vector.tensor_copy(wseed[:], pwm[0:1, 0:1])

                # ======== conv stack: 6 banded layers on [36, 6] ========
                # high priority: the 6 relu round-trips are an ~8us
                # latency chain feeding tanh(h0); they must win the
                # scheduler race against the bulk L0/L1 matmuls.
                hp = tc.high_priority()
                hp.__enter__()
                v = pkb[0:NS, PB_V:PB_V + NZ]
                for i in range(6):
                    b = i * 3 * NS
                    pm = psC.tile([NS, NZ], F32)
                    nc.tensor.matmul(pm[:, 0:6], ctb[:, b + 36:b + 72],
                                     v[:, 0:6], start=True, stop=False,
                                     skip_group_check=True)
                    nc.tensor.matmul(pm[:, 1:6], ctb[:, b:b + 36],
                                     v[:, 0:5], start=False, stop=False,
                                     skip_group_check=True)
                    nc.tensor.matmul(pm[:, 0:5], ctb[:, b + 72:b + 108],
                                     v[:, 1:6], start=False, stop=True,
                                     skip_group_check=True)
                    nv = sp.tile([NS, NZ], BF16)
                    nc.scalar.activation(nv[:], pm[:], AF.Relu,
                                         bias=pkf[:, i:i + 1])
                    v = nv[:]
                hp.__exit__(None, None, None)

                a0t = []
                a0eng = [nc.sync, nc.scalar, nc.gpsimd]
                row = 0
                for p, gch in enumerate(A0P):
                    wt = wp.tile([128, gch * S], BF16)
                    a0eng[p].dma_start(
                        out=wt[:].rearrange("p (j e) -> p j e", j=gch),
                        in_=a0_d[row * 128:(row + gch) * 128, :]
                        .rearrange("(j p) e -> p j e", p=128))
                    a0t.append(wt)
                    row += gch
                a1eng = [nc.sync, nc.scalar, nc.gpsimd, nc.sync,
                         nc.scalar, nc.gpsimd, nc.sync, nc.gpsimd]

                def stage_a1(p):
                    wt = wp.tile([128, G1 * S], BF16)
                    a1eng[p].dma_start(
                        out=wt[:].rearrange("p (j e) -> p j e", j=G1),
                        in_=a1_d[p * G1 * 128:(p + 1) * G1 * 128, :]
                        .rearrange("(j p) e -> p j e", p=128))
                    return wt

                a1t = [stage_a1(p) for p in range(5)]

                # ======== L0: h0_blk = tanh(xn @ A0 + b0_blk) [1,512] ===
                # conv tail FIRST in the accumulation: the x-chunk
                # matmuls then chain behind it in the same PSUM region,
                # so the greedy scheduler cannot hoist them ahead of the
                # conv relu chain (which would starve conv of PE slots).
                hp = tc.high_priority(offset=100000)
                hp.__enter__()
                ph0 = psA.tile([1, S], F32)
                for z in range(NZ):
                    nc.tensor.matmul(ph0[:], v[:, z:z + 1],
                                     wtail[:, z * S:(z + 1) * S],
                                     start=(z == 0), stop=False)
                c = 0
                for p, gch in enumerate(A0P):
                    for j in range(gch):
                        nc.tensor.matmul(ph0[:], pkb[:, c:c + 1],
                                         a0t[p][:, j * S:(j + 1) * S],
                                         start=False, stop=False)
                        c += 1
                nc.tensor.matmul(ph0[:], one_b, pkbias[0:1, BB_B0:BB_B0 + S],
                                 start=False, stop=True)
                h0s = sp.tile([1, S], BF16)
                nc.scalar.activation(h0s[:], ph0[:], AF.Tanh)

                # ======== AllGather h0 blocks -> full h0 [4096] =========
                ccin = dr.tile([S], BF16)
                nc.scalar.dma_start(out=ccin[:], in_=h0s[:])
                ccout = dr.tile([H], BF16)
                if fake_gather:
                    # stand-in: a gpsimd DRAM bounce keeps the dependency
                    # chain + ring shape identical to the real AllGather
                    nc.gpsimd.dma_start(out=ccout[0:S], in_=ccin[:])
                else:
                    nc.gpsimd.collective_compute(
                        "AllGather", mybir.AluOpType.bypass,
                        replica_groups=[list(range(NCORES))],
                        ins=[ccin[:].opt()], outs=[ccout[:].opt()])

                a2t = sp.tile([128, (S // 128) * ACTD], BF16)
                nc.sync.dma_start(
                    out=a2t[:].rearrange("p (j e) -> p j e", j=S // 128),
                    in_=a2_d[:, :].rearrange("(j p) e -> p j e", p=128))

                # gathered h0 comes back as one row (a single descriptor)
                ccrow = sp.tile([1, H], BF16)
                nc.scalar.dma_start(out=ccrow[:], in_=ccout[:].unsqueeze(0))
                hp.__exit__(None, None, None)

                # last three a1 pieces: WAW-gated on ccrow so the gather
                # round-trip's transfers outrank them in the DMA queues
                for p in range(5, 8):
                    wq = wqs[p - 5]
                    tq = wq.tile([1, 1], BF16, name="tq")
                    # p5 rides the gather window's idle DMA slots; the
                    # last two wait for the returned row
                    gate = h0s if p == 5 else ccrow
                    nc.vector.tensor_copy(tq[:], gate[0:1, p:p + 1])
                    wt = wq.tile([128, G1 * S], BF16, name="tq")
                    a1eng[p].dma_start(
                        out=wt[:].rearrange("p (j e) -> p j e", j=G1),
                        in_=a1_d[p * G1 * 128:(p + 1) * G1 * 128, :]
                        .rearrange("(j p) e -> p j e", p=128))
                    a1t.append(wt)

                # ======== PE p-state bridge across the gather window ====
                # short self-paced PE<->DVE chain pinned to h0s keeps the
                # tensor clock ramped while the gather flies
                pbr = psT.tile([1, 64], F32, name="ptr2")
                brb = sp.tile([1, 1], BF16, name="brb")
                nc.vector.tensor_copy(brb[:], h0s[0:1, 0:1])
                for k in range(9):
                    nc.tensor.matmul(pbr[:], brb[:],
                                     pkbias[0:1, BB_B0:BB_B0 + 64],
                                     start=(k == 0), stop=(k == 8),
                                     skip_group_check=True)
                    if k < 8:
                        brb = sp.tile([1, 1], BF16, name="brb")
                        nc.vector.tensor_copy(brb[:], pbr[0:1, 0:1])

                # ======== L1 + h0 redistribution ========================
                # transposes turn ccrow into [128,1] K-chunk columns 8 at
                # a time; each L1 group consumes its a1 piece as it lands.
                h0g = sp.tile([128, KC1], BF16)
                for t in range(4):
                    ptr2 = psT.tile([128, 8], F32, name="ptr2")
                    for j in range(8):
                        cc = t * 8 + j
                        nc.tensor.matmul(
                            ptr2[:, j:j + 1],
                            ccrow[:, cc * 128:(cc + 1) * 128], one_b,
                            start=True, stop=True)
                    nc.scalar.copy(h0g[:, t * 8:(t + 1) * 8], ptr2[:])
                ph1 = psA.tile([1, S], F32)
                for p in range(8):
                    for j in range(G1):
                        cc = p * G1 + j
                        nc.tensor.matmul(ph1[:], h0g[:, cc:cc + 1],
                                         a1t[p][:, j * S:(j + 1) * S],
                                         start=(cc == 0), stop=False)
                nc.tensor.matmul(ph1[:], one_b, pkbias[0:1, BB_B1:BB_B1 + S],
                                 start=False, stop=True)
                h1s = sp.tile([1, S], BF16)
                nc.scalar.activation(h1s[:], ph1[:], AF.Tanh)

                # ======== L2 partial: y_k = h1_blk @ A2_blk + b'/8 ======
                pth = psB.tile([128, S // 128], F32)
                for cc in range(S // 128):
                    nc.tensor.matmul(pth[:, cc:cc + 1],
                                     h1s[:, cc * 128:(cc + 1) * 128], one_b,
                                     start=True, stop=True)
                h1g = sp.tile([128, S // 128], BF16)
                nc.scalar.copy(h1g[:], pth[:])
                py = psA.tile([1, ACTD], F32)
                for cc in range(S // 128):
                    nc.tensor.matmul(py[:], h1g[:, cc:cc + 1],
                                     a2t[:, cc * ACTD:(cc + 1) * ACTD],
                                     start=(cc == 0), stop=False)
                nc.tensor.matmul(py[:], one_b,
                                 pkbias[0:1, BB_B2:BB_B2 + ACTD],
                                 start=False, stop=True)
                ys = sp.tile([1, ACTD], F32)
                nc.scalar.copy(ys[:], py[:])
                nc.scalar.dma_start(out=y_d[:, :], in_=ys[:])

    nc.compile()
    return nc


def _conv_matrix(w: np.ndarray) -> np.ndarray:
    """[216,216] dense matrix of a 3x3x3 SAME cross-correlation on a
    6x6x6 grid: C[o, i] such that y.flat = C @ v.flat."""
    w = np.asarray(w, dtype=np.float32).reshape(3, 3, 3)
    C = np.zeros((VOX, VOX), dtype=np.float32)
    idx = np.arange(6)
    for dz in (-1, 0, 1):
        for dy in (-1, 0, 1):
            for dx in (-1, 0, 1):
                zo, zi = idx[max(0, -dz):6 - max(0, dz)], idx[max(0, dz):6 - max(0, -dz)]
                yo, yi = idx[max(0, -dy):6 - max(0, dy)], idx[max(0, dy):6 - max(0, -dy)]
                xo, xi = idx[max(0, -dx):6 - max(0, dx)], idx[max(0, dx):6 - max(0, -dx)]
                o = (zo[:, None, None] * 36 + yo[None, :, None] * 6 + xo[None, None, :]).ravel()
                i = (zi[:, None, None] * 36 + yi[None, :, None] * 6 + xi[None, None, :]).ravel()
                C[o, i] = w[dz + 1, dy + 1, dx + 1]
    return C


def make_in_maps(inputs: dict) -> list[dict]:
    """Host-side layout prep + sharding: fold normalization into A0,
    out_scale/shift into A2, pre-transpose weights, build banded conv
    matrices, quantize everything to bf16."""
    import ml_dtypes
    f = np.float32
    bf = ml_dtypes.bfloat16
    x = np.asarray(inputs["x"], f)
    W0, b0 = np.asarray(inputs["W0"], f), np.asarray(inputs["b0"], f)
    W1, b1 = np.asarray(inputs["W1"], f), np.asarray(inputs["b1"], f)
    W2, b2 = np.asarray(inputs["W2"], f), np.asarray(inputs["b2"], f)
    in_shift = np.asarray(inputs["in_shift"], f)
    in_scale = np.asarray(inputs["in_scale"], f)
    out_shift = np.asarray(inputs["out_shift"], f)
    out_scale = np.asarray(inputs["out_scale"], f)

    sc = (1.0 / (in_scale.astype(np.float64) + 1e-8)).astype(f)       # [2264]
    A0 = (W0 * sc[None, :]).T.astype(f)                               # [2264, 4096]
    bias0 = (b0 - (in_shift * sc) @ W0.T).astype(f)                   # [4096]
    A1 = W1.T.astype(f)                                               # [4096, 4096]
    A2 = (W2.T * out_scale[None, :]).astype(f)                        # [4096, 32]
    bias2 = ((b2 * out_scale + out_shift) / NCORES).astype(f)         # [32]

    # banded conv: per layer i and z-offset dz, M_dz [36,36] stored
    # transposed (lhsT layout): ctb[s_in, (i*3 + dz+1)*36 + s_out]
    ctb = np.zeros((NS, CTW), f)
    for i in range(6):
        C = _conv_matrix(inputs[f"cw{i}"])
        for dzi, dz in enumerate((-1, 0, 1)):
            M = C[1 * NS:2 * NS, (1 + dz) * NS:(2 + dz) * NS]         # [out, in]
            ctb[:, (i * 3 + dzi) * NS:(i * 3 + dzi + 1) * NS] = M.T
    cb = np.array([np.asarray(inputs[f"cb{i}"], f).ravel()[0]
                   for i in range(6)], f)
    pkf = np.broadcast_to(cb[None, :], (NS, 6)).astype(f)

    xh = x.ravel()[:XH]
    v0 = x.ravel()[XH:]
    pkb = np.zeros((128, PB_W), f)
    pkb[:, PB_X:PB_X + KC0] = xh.reshape(KC0, 128).T
    pkb[0:NS, PB_V:PB_V + NZ] = v0.reshape(NZ, NS).T

    # A0 tail columns regrouped by z-slice: wtail[s, z*S+n] = A0[2048+z*36+s, n]
    def wtail_for(blk):
        t = A0[XH:OBS, blk].reshape(NZ, NS, S)
        return np.ascontiguousarray(t.transpose(1, 0, 2).reshape(NS, NZ * S))

    in_maps = []
    for k in range(NCORES):
        blk = slice(k * S, (k + 1) * S)
        pkbias = np.zeros((1, BB_W), f)
        pkbias[0, BB_ONE] = 1.0
        pkbias[0, BB_B0:BB_B0 + S] = bias0[blk]
        pkbias[0, BB_B1:BB_B1 + S] = b1[blk]
        pkbias[0, BB_B2:BB_B2 + ACTD] = bias2
        in_maps.append(dict(
            ctb=ctb.astype(bf), pkf=pkf, pkb=pkb.astype(bf),
            pkbias=pkbias.astype(bf),
            a0=np.ascontiguousarray(A0[:XH, blk]).astype(bf),
            wtail=wtail_for(blk).astype(bf),
            a1=np.ascontiguousarray(A1[:, blk]).astype(bf),
            a2=np.ascontiguousarray(A2[blk, :]).astype(bf),
        ))
    return in_maps


_NC_CACHE: dict = {}


def kernel(**inputs) -> np.ndarray:
    if "nc" not in _NC_CACHE:
        _NC_CACHE["nc"] = build_nc(reps=1)
    nc = _NC_CACHE["nc"]
    in_maps = make_in_maps(inputs)
    res = bass_utils.run_bass_kernel_spmd(nc, in_maps,
                                          core_ids=list(range(NCORES)))
    y = np.sum([res.results[k]["y"] for k in range(NCORES)], axis=0)
    return y.astype(np.float32)
